# revision 42
# baseline (speedup 1.0000x reference)
"""PSRoIPool (position-sensitive ROI pooling) for Trainium2, 8 NeuronCores.

Problem (hardcoded):
  features [4, 392, 128, 128] f32, rois [512, 5] f32 (batch, x1, y1, x2, y2)
  out [512, 8, 7, 7] f32;  C = C_out(8) * 7 * 7;  spatial_scale = 1/16.

Sharding: by output channel c_out (8 cores). Core k owns feature channels
[49k, 49k+49) of every image (1/8 of the features, read exactly once as
bf16) and computes out[:, k, :, :] for ALL 512 rois.

Algorithm (per core):
  - rois stable-sorted by batch -> exactly 4 windows of 128 rois. A window
    spanning a batch boundary is computed as 2 PSUM-accumulated matmul
    passes whose H-masks are zeroed outside their roi range (host-side).
  - stage 1 (PE, bf16): u[r, pw, w] = sum_h Mh[h, r; ph] * F[h, c_s, w]
    per (window, ph); H-mask is the stationary operand.
  - stage 2 (DVE, one fused pass): custom DVE op MASKED_PREFIX_ANT computes
    P[r, t] = prefix-sum of u[r, t] * Mw[r, t] over the flat (pw, w) axis.
    Bin sums drop out as strided differences at the 128-column boundaries:
      bins[r, ph, 0]    = P[r, 127]
      bins[r, ph, pw>0] = P[r, 128(pw+1)-1] - P[r, 128 pw - 1]
  - out = bins * recip_area (recip precomputed on host, 0 for empty bins).

All masks + reciprocal areas are precomputed on the host and DMAed (they
depend only on the rois and are identical on all 8 cores). Masks are fp8
(0/1 exact, ~1.15 MB total); features are dtype-split: 37 of 49 channels
bf16 + 12 channels fp8 (adds ~1.3e-2 rel err vs the 2e-2 gate, saves 12%
of feature DMA). All input DMAs are consolidated into 5 transfers per
iteration (HWDGE per-DMA overhead ~0.7us dominates small transfers), and
the 4 window outputs leave in a single DMA.
"""

import numpy as np
from contextlib import ExitStack

try:
    import ml_dtypes
    _BF16 = ml_dtypes.bfloat16
    _FP8 = ml_dtypes.float8_e4m3
except ImportError:  # pragma: no cover
    import jax.numpy as _jnp
    _BF16 = _jnp.bfloat16
    _FP8 = _jnp.float8_e4m3

N_IMG, C_FULL, H, W = 4, 392, 128, 128
R = 512
P = 7  # OUT_SIZE == GROUP
C_OUT = 8
C_SLAB = P * P  # 49 channels per core
SCALE = np.float32(0.0625)
NCORES = 8
RW = 128  # rois per window
NWIN = R // RW

OP_NAME = "MASKED_PREFIX_ANT"

# channels stored as fp8 (pw 0-3 of ph 0/2/4): the (0,4) matmul split of each
# ph-pair group. err contribution ~2.6e-2 * sqrt(12/49) ~ 1.3e-2 < 2e-2 gate.
FP8_CS = [0, 1, 2, 3, 14, 15, 16, 17, 28, 29, 30, 31]
BF_CS = [c for c in range(C_SLAB) if c not in FP8_CS]  # 37 channels
N_F8 = len(FP8_CS)
N_BF = len(BF_CS)


# ------------------------------------------------------------- custom DVE op
def _get_custom_op():
    """out[p, k] = sum_{j<=k} in0[p, j] * in1[p, j] (fp32 internal state).

    Registered by appending to concourse.dve_ops.OPS (the per-NEFF DVE
    table is generated from OPS by bass_utils.dve_table_for_ops)."""
    import concourse.dve_ops as D

    for op in D.OPS:
        if op.name == OP_NAME:
            return op

    from concourse.dve_spec import Spec, scan, AluOp, Src0, Src1, lower, _has_src1
    from concourse.dve_uop import DveOpSpec

    def ref(in0, in1, s0, s1, imm2):
        p = in0.astype(np.float32) * in1.astype(np.float32)
        return np.add.accumulate(p, axis=-1).astype(np.float32)

    spec = Spec(body=scan(AluOp.ADD, Src0 * Src1), reference=ref)
    row = max(D._SUB_OPCODE_FOR_NAME.values()) + 1
    assert row < 0x20
    shas = {}
    for ver in ("v3", "v4"):
        s = DveOpSpec(name=OP_NAME, opcode=row,
                      uops=lower(spec, ver=ver), rd1_en=_has_src1(spec))
        shas[ver] = s.sha(ver)
    op = D.DveOp(OP_NAME, spec, subdim=False, uops_sha=shas)
    D.OPS.append(op)
    D.CUSTOM_DVE_SPECS[op.name] = op.spec
    D._SUB_OPCODE_FOR_NAME[op.name] = row
    return op


# ---------------------------------------------------------------- host math
def _bounds(rois_sorted: np.ndarray):
    """Bit-exact f32 mirror of the reference coordinate transform."""
    r = rois_sorted.astype(np.float32)
    one = np.float32(1.0)
    rsw = np.round(r[:, 1]) * SCALE
    rsh = np.round(r[:, 2]) * SCALE
    rew = (np.round(r[:, 3]) + one) * SCALE
    reh = (np.round(r[:, 4]) + one) * SCALE
    roi_w = np.maximum(rew - rsw, np.float32(0.1))
    roi_h = np.maximum(reh - rsh, np.float32(0.1))
    bin_w = (roi_w / np.float32(P)).astype(np.float32)
    bin_h = (roi_h / np.float32(P)).astype(np.float32)
    p = np.arange(P, dtype=np.float32)
    hs = np.clip(np.floor(p[None, :] * bin_h[:, None] + rsh[:, None]), 0, H)
    he = np.clip(np.ceil((p[None, :] + one) * bin_h[:, None] + rsh[:, None]), 0, H)
    ws = np.clip(np.floor(p[None, :] * bin_w[:, None] + rsw[:, None]), 0, W)
    we = np.clip(np.ceil((p[None, :] + one) * bin_w[:, None] + rsw[:, None]), 0, W)
    return hs, he, ws, we  # [R, P] f32 (integer-valued)


def _plan(batch: np.ndarray):
    """Pack rois into NWIN windows of RW, minimising matmul streams: pull
    whole-window pure-batch groups first, then chunk the remainders.
    Returns (order [R], pieces: per-window ((b, c0, c1), ...))."""
    groups = [np.nonzero(batch == b)[0] for b in range(N_IMG)]
    pure, rem = [], []
    for idx in groups:
        n_pure = len(idx) // RW
        for i in range(n_pure):
            pure.append(idx[i * RW:(i + 1) * RW])
        rem.append(idx[n_pure * RW:])
    rem = np.concatenate([r for r in rem if len(r)]) if any(
        len(r) for r in rem) else np.empty(0, np.int64)
    order = np.concatenate(pure + ([rem] if len(rem) else []))
    assert len(order) == R
    sb = batch[order]
    pieces = []
    for w in range(NWIN):
        seg = sb[w * RW:(w + 1) * RW]
        ps = []
        start = 0
        for i in range(1, RW + 1):
            if i == RW or seg[i] != seg[start]:
                ps.append((int(seg[start]), start, i))
                start = i
        pieces.append(tuple(ps))
    return order, tuple(pieces)


def _prep(rois: np.ndarray):
    batch = rois[:, 0].astype(np.int32)
    order, pieces = _plan(batch)
    rs = rois[order]
    hs, he, ws, we = _bounds(rs)

    harange = np.arange(H, dtype=np.float32)
    warange = np.arange(W, dtype=np.float32)

    # hmask streams: one per (window, piece): [h, ph, r] zeroed outside piece
    hm_list = []
    for w, ps in enumerate(pieces):
        sl = slice(w * RW, (w + 1) * RW)
        hsw, hew = hs[sl], he[sl]  # [RW, P]
        m = ((harange[:, None, None] >= hsw.T[None, :, :])
             & (harange[:, None, None] < hew.T[None, :, :]))  # [H, P, RW]
        for (b, c0, c1) in ps:
            mm = np.zeros((H, P, RW), dtype=np.float32)
            mm[:, :, c0:c1] = m[:, :, c0:c1]
            hm_list.append(mm.reshape(H, P * RW))
    hmask = np.asarray(np.stack(hm_list), dtype=_FP8)  # [NS, 128, 896]

    # W-interval mask per window: [r, pw, w]
    mw = ((warange[None, None, :] >= ws[:, :, None])
          & (warange[None, None, :] < we[:, :, None])).astype(np.float32)
    mww = np.asarray(mw.reshape(NWIN, RW, P * W), dtype=_FP8)

    # open-ended masks for the Pool (gpsimd) scan path on window 0:
    # (w >= max(bound, 1)) so the (u + state) * m recurrence resets at each
    # 128-col channel start; the dropped w=0 term is re-added as zz * u[.,0]
    w0 = slice(0, RW)
    ma = (warange[None, None, :] >= np.maximum(ws[w0], 1.0)[:, :, None])
    mb = (warange[None, None, :] >= np.maximum(we[w0], 1.0)[:, :, None])
    mao = np.asarray(ma.reshape(RW, P * W).astype(np.float32), dtype=_BF16)
    mbo = np.asarray(mb.reshape(RW, P * W).astype(np.float32), dtype=_BF16)
    zz = ((ws[w0] == 0).astype(np.float32)
          - (we[w0] == 0).astype(np.float32))  # [RW, P] per pw
    # zz indexed by cs = ph*7+pw -> tile [RW, 49]
    zz49 = np.repeat(zz[:, None, :], P, axis=1).reshape(RW, C_SLAB)
    zz49 = zz49.astype(np.float32)

    # reciprocal area, 0 where empty  [NWIN, RW, 49] (cs = ph*7+pw)
    ah = he - hs  # [R, P]
    aw = we - ws
    area = ah[:, :, None] * aw[:, None, :]  # [R, ph, pw]
    recip = np.where(area > 0, np.float32(1.0) / np.maximum(area, 1.0),
                     np.float32(0.0)).astype(np.float32)
    recip = recip.reshape(NWIN, RW, C_SLAB)

    return order, pieces, hmask, mww, recip, mao, mbo, zz49


# ---------------------------------------------------------------- device IR
def build_program(pieces, repeat=1, dma_once=False, tiny_scan=False,
                  fat_scan=False, first_piece_only=False, pool_gis=()):
    """pool_gis: window-0 group indices whose scans would run on GPSIMD
    (Pool) instead of DVE. MUST stay () — the neuronx backend cannot
    compile tensor_tensor_scan on the Pool engine (verified empirically:
    even a minimal Pool-scan program fails in the compile hook), which is
    why stage 2 uses the custom DVE op in the first place."""
    import concourse.bass as bass
    import concourse.tile as tile
    from concourse import bacc, mybir

    f32 = mybir.dt.float32
    bf16 = mybir.dt.bfloat16
    fp8 = mybir.dt.float8e4
    Alu = mybir.AluOpType
    op = _get_custom_op()

    nstream = sum(len(ps) for ps in pieces)

    nc = bacc.Bacc("TRN2", target_bir_lowering=False, debug=False,
                   num_devices=NCORES)

    # feature slab split by storage dtype; [N, H, C, W] so each partition
    # line is C*W contiguous bytes
    fslab_bf = nc.dram_tensor("fslab_bf", [N_IMG, H, N_BF, W], bf16,
                              kind="ExternalInput").ap()
    fslab_f8 = nc.dram_tensor("fslab_f8", [N_IMG, H, N_F8, W], fp8,
                              kind="ExternalInput").ap()
    # hmask streams and w-masks share shape [128, 896] fp8: one fused tensor
    # [nstream + NWIN, 128, 896] -> one DMA (HWDGE churn dominates small DMAs)
    hmw = nc.dram_tensor("hmw", [nstream + NWIN, H, P * RW], fp8,
                         kind="ExternalInput").ap()
    # open-ended masks (mao; mbo) + zz edge term for the Pool scan of win 0
    mab = nc.dram_tensor("mab", [2, RW, P * W], bf16,
                         kind="ExternalInput").ap()
    zz = nc.dram_tensor("zz", [RW, C_SLAB], f32, kind="ExternalInput").ap()
    recip = nc.dram_tensor("recip", [NWIN, RW, C_SLAB], f32,
                           kind="ExternalInput").ap()
    out = nc.dram_tensor("out", [R, C_SLAB], f32, kind="ExternalOutput").ap()

    with tile.TileContext(nc) as tc, ExitStack() as ctx:
        fpool = ctx.enter_context(tc.tile_pool(name="fs", bufs=2))
        mpool = ctx.enter_context(tc.tile_pool(name="masks", bufs=2))
        spool = ctx.enter_context(tc.tile_pool(name="scratch", bufs=2))
        upool = ctx.enter_context(tc.tile_pool(name="ucopy", bufs=2))
        bpool = ctx.enter_context(tc.tile_pool(name="bins", bufs=2))
        opool = ctx.enter_context(tc.tile_pool(name="outw", bufs=2))
        # u2 tiles are [128, 14, 128] f32 = 3.5 PSUM banks; 2 bufs = 7 of 8
        psum = ctx.enter_context(tc.tile_pool(name="ps", bufs=2, space="PSUM"))

        for _rep in range(repeat):
            if _rep == 0 or not dma_once:
                # one consolidated DMA per tensor: DMA-queue (HWDGE) churn
                # dominates small transfers, so batch everything
                fsall = fpool.tile([128, N_IMG, N_BF, W], bf16, tag="fsall")
                nc.sync.dma_start(
                    out=fsall[:],
                    in_=bass.AP(tensor=fslab_bf.tensor, offset=0,
                                ap=[[N_BF * W, H], [H * N_BF * W, N_IMG],
                                    [W, N_BF], [1, W]]))
                fsall8 = fpool.tile([128, N_IMG, N_F8, W], fp8, tag="fsall8")
                nc.sync.dma_start(
                    out=fsall8[:],
                    in_=bass.AP(tensor=fslab_f8.tensor, offset=0,
                                ap=[[N_F8 * W, H], [H * N_F8 * W, N_IMG],
                                    [W, N_F8], [1, W]]))
                fs = [(fsall[:, b], fsall8[:, b]) for b in range(N_IMG)]

            hmall = mpool.tile([128, nstream + NWIN, P * RW], fp8, tag="hmall")
            nc.sync.dma_start(
                out=hmall[:],
                in_=bass.AP(tensor=hmw.tensor, offset=0,
                            ap=[[P * RW, H], [H * P * RW, nstream + NWIN],
                                [1, P * RW]]))
            mwall = hmall  # w-masks live at slots [nstream, nstream+NWIN)
            rcall = mpool.tile([128, NWIN, C_SLAB], f32, tag="rcall")
            nc.sync.dma_start(
                out=rcall[:],
                in_=bass.AP(tensor=recip.tensor, offset=0,
                            ap=[[C_SLAB, RW], [RW * C_SLAB, NWIN],
                                [1, C_SLAB]]))
            if pool_gis:
                mabt = mpool.tile([128, 2, P * W], bf16, tag="mabt")
                nc.sync.dma_start(
                    out=mabt[:],
                    in_=bass.AP(tensor=mab.tensor, offset=0,
                                ap=[[P * W, RW], [RW * P * W, 2],
                                    [1, P * W]]))
                zzt = mpool.tile([128, C_SLAB], f32, tag="zzt")
                nc.sync.dma_start(
                    out=zzt[:],
                    in_=bass.AP(tensor=zz.tensor, offset=0,
                                ap=[[C_SLAB, RW], [1, C_SLAB]]))
            outw = opool.tile([128, NWIN, C_SLAB], f32, tag="outw")

            sidx = 0
            for win in range(NWIN):
                ps = pieces[win]
                hm_off = []
                for _ in ps:
                    hm_off.append(hmall.offset + sidx * P * RW)
                    sidx += 1
                mw_off = mwall.offset + (nstream + win) * P * W
                bins = bpool.tile([128, C_SLAB], f32, tag="bins")

                # ph groups: pairs (0,1),(2,3),(4,5) + single (6,). One fused
                # scan per group over the PSUM tile; matmul column-splits are
                # PSUM-bank-aligned AND split at the ph boundary (row 7):
                # rows 0-6 use ph a's H-mask, rows 7-13 ph b's.
                for gi, grp in enumerate(((0, 1), (2, 3), (4, 5), (6,))):
                    pa = grp[0]
                    nrow = 7 * len(grp)
                    base = pa * P  # contiguous channels base+0 .. base+nrow-1
                    u = psum.tile([128, 2 * P, W], f32, tag="u2")
                    if len(grp) == 2:
                        splits = ((0, 4, grp[0]), (4, 7, grp[0]),
                                  (7, 8, grp[1]), (8, 12, grp[1]),
                                  (12, 14, grp[1]))
                    else:
                        splits = ((0, 4, grp[0]), (4, 7, grp[0]))
                    g = pa // 2
                    ps_mm = ps[:1] if first_piece_only else ps
                    for (n0, n1, p_) in splits:
                        # rhs rows in the dtype-split storage: the (0,4)
                        # split of each pair group reads the fp8 slab
                        if len(grp) == 2 and n0 == 0:
                            def rhs_of(b):
                                return bass.AP(
                                    tensor=fsall8.tensor,
                                    offset=fsall8.offset
                                    + (b * N_F8 + g * 4) * W,
                                    ap=[fsall8.ap[0], [W, 4], [1, W]])
                        else:
                            row0 = (g * 10 + n0 - 4) if len(grp) == 2 \
                                else (30 + n0)
                            def rhs_of(b, row0=row0, nr=n1 - n0):
                                return bass.AP(
                                    tensor=fsall.tensor,
                                    offset=fsall.offset
                                    + (b * N_BF + row0) * W,
                                    ap=[fsall.ap[0], [W, nr], [1, W]])
                        for i, (b, c0, c1) in enumerate(ps_mm):
                            nc.tensor.matmul(
                                out=u[:, n0:n1, :],
                                lhsT=bass.AP(
                                    tensor=hmall.tensor,
                                    offset=hm_off[i] + p_ * RW,
                                    ap=[hmall.ap[0], [1, RW]]),
                                rhs=rhs_of(b),
                                start=(i == 0), stop=(i == len(ps_mm) - 1))
                    if (win == 0 and gi in pool_gis
                            and not (tiny_scan or fat_scan)):
                        # offloaded group: scans on the (otherwise idle)
                        # GPSIMD. ACT first copies u PSUM->SBUF f32 so the
                        # PSUM buffer is released at ACT speed and the slow
                        # Pool scans never stall the PE's PSUM rotation.
                        # Two open-ended (u + state) * m recurrences per ph;
                        # bin = A_end - B_end + zz * u[., 0] at the fixed
                        # per-channel end columns (state resets at each
                        # 128-col start since the masks are 0 at w=0).
                        ucp = upool.tile([128, 2 * P * W], f32, tag="ucp")
                        u_flat = bass.AP(tensor=u.tensor, offset=u.offset,
                                         ap=[u.ap[0], [1, nrow * W]])
                        nc.scalar.copy(ucp[:, 0:nrow * W], u_flat)
                        preA = spool.tile([128, 2 * P * W], f32, tag="preA")
                        preB = spool.tile([128, 2 * P * W], f32, tag="preB")
                        for phi in range(len(grp)):
                            u_sl = bass.AP(tensor=ucp.tensor,
                                           offset=ucp.offset + phi * P * W,
                                           ap=[ucp.ap[0], [1, P * W]])
                            for pre_t, moff in ((preA, 0), (preB, P * W)):
                                nc.gpsimd.tensor_tensor_scan(
                                    out=bass.AP(
                                        tensor=pre_t.tensor,
                                        offset=pre_t.offset + phi * P * W,
                                        ap=[pre_t.ap[0], [1, P * W]]),
                                    data0=u_sl,
                                    data1=bass.AP(
                                        tensor=mabt.tensor,
                                        offset=mabt.offset + moff,
                                        ap=[mabt.ap[0], [1, P * W]]),
                                    initial=0.0,
                                    op0=Alu.add, op1=Alu.mult)
                        cs0 = base
                        hiA = bass.AP(tensor=preA.tensor,
                                      offset=preA.offset + W - 1,
                                      ap=[preA.ap[0], [W, nrow]])
                        hiB = bass.AP(tensor=preB.tensor,
                                      offset=preB.offset + W - 1,
                                      ap=[preB.ap[0], [W, nrow]])
                        nc.gpsimd.tensor_tensor(out=bins[:, cs0:cs0 + nrow],
                                                in0=hiA, in1=hiB,
                                                op=Alu.subtract)
                        u0 = bass.AP(tensor=ucp.tensor, offset=ucp.offset,
                                     ap=[ucp.ap[0], [W, nrow]])
                        zsl = bass.AP(tensor=zzt.tensor,
                                      offset=zzt.offset + cs0,
                                      ap=[zzt.ap[0], [1, nrow]])
                        ztmp = spool.tile([128, 2 * P], f32, tag="ztmp")
                        nc.gpsimd.tensor_tensor(out=ztmp[:, 0:nrow],
                                                in0=u0, in1=zsl, op=Alu.mult)
                        nc.gpsimd.tensor_tensor(out=bins[:, cs0:cs0 + nrow],
                                                in0=bins[:, cs0:cs0 + nrow],
                                                in1=ztmp[:, 0:nrow],
                                                op=Alu.add)
                        continue
                    pre = spool.tile([128, (4 if fat_scan else 2) * P * W],
                                     f32, tag="pre")
                    if fat_scan:
                        # DVE-rate probe: stream u through the scan twice via
                        # a [0,2] broadcast — results in the first copy are
                        # unchanged, DVE scan time doubles.
                        u_fat = bass.AP(tensor=u.tensor, offset=u.offset,
                                        ap=[u.ap[0], [0, 2], [1, nrow * W]])
                        m_fat = bass.AP(tensor=mwall.tensor, offset=mw_off,
                                        ap=[mwall.ap[0],
                                            [0, 4 if len(grp) == 2 else 2],
                                            [1, P * W]])
                        nc.vector._custom_dve(op, out=pre[:, 0:2 * nrow * W],
                                              in0=u_fat, in1=m_fat)
                    elif tiny_scan:
                        # timing probe: scan only the 14 columns that are
                        # later read (positions k*W + W-1), deps preserved
                        pre_s = bass.AP(tensor=pre.tensor,
                                        offset=pre.offset + W - 1,
                                        ap=[pre.ap[0], [W, nrow]])
                        u_s = bass.AP(tensor=u.tensor, offset=u.offset + W - 1,
                                      ap=[u.ap[0], [W, nrow]])
                        if len(grp) == 2:
                            m_s = bass.AP(tensor=mwall.tensor,
                                          offset=mw_off + W - 1,
                                          ap=[mwall.ap[0], [0, 2], [W, P]])
                        else:
                            m_s = bass.AP(tensor=mwall.tensor,
                                          offset=mw_off + W - 1,
                                          ap=[mwall.ap[0], [W, P]])
                        nc.vector._custom_dve(op, out=pre_s, in0=u_s, in1=m_s)
                    else:
                        u_flat = bass.AP(tensor=u.tensor, offset=u.offset,
                                         ap=[u.ap[0], [1, nrow * W]])
                        if len(grp) == 2:
                            # mask streams twice (same [r, pw, w] mask per ph)
                            m_in = bass.AP(tensor=mwall.tensor, offset=mw_off,
                                           ap=[mwall.ap[0], [0, 2],
                                               [1, P * W]])
                        else:
                            m_in = bass.AP(tensor=mwall.tensor, offset=mw_off,
                                           ap=[mwall.ap[0], [1, P * W]])
                        nc.vector._custom_dve(op, out=pre[:, 0:nrow * W],
                                              in0=u_flat, in1=m_in)
                    cs0 = base
                    nc.scalar.copy(bins[:, cs0:cs0 + 1], pre[:, W - 1:W])
                    hi = bass.AP(tensor=pre.tensor, offset=pre.offset + 2 * W - 1,
                                 ap=[pre.ap[0], [W, nrow - 1]])
                    lo = bass.AP(tensor=pre.tensor, offset=pre.offset + W - 1,
                                 ap=[pre.ap[0], [W, nrow - 1]])
                    nc.gpsimd.tensor_tensor(out=bins[:, cs0 + 1:cs0 + nrow],
                                            in0=hi, in1=lo, op=Alu.subtract)

                rct = bass.AP(tensor=rcall.tensor,
                              offset=rcall.offset + win * C_SLAB,
                              ap=[rcall.ap[0], [1, C_SLAB]])
                ow = bass.AP(tensor=outw.tensor,
                             offset=outw.offset + win * C_SLAB,
                             ap=[outw.ap[0], [1, C_SLAB]])
                nc.gpsimd.tensor_tensor(out=ow, in0=bins[:], in1=rct,
                                        op=Alu.mult)

            # single consolidated output DMA: SBUF [r, win, cs] -> out rows
            nc.sync.dma_start(
                out=bass.AP(tensor=out.tensor, offset=0,
                            ap=[[C_SLAB, RW], [RW * C_SLAB, NWIN],
                                [1, C_SLAB]]),
                in_=outw[:])

    nc.compile()
    return nc


_PROG_CACHE = {}


def _get_program(pieces, repeat=1):
    key = (pieces, repeat)
    if key not in _PROG_CACHE:
        _PROG_CACHE[key] = build_program(pieces, repeat=repeat)
    return _PROG_CACHE[key]


# ---------------------------------------------------------------- entrypoint
def make_in_maps(features: np.ndarray, hmask, mww, recip, mao, mbo, zz49):
    """Per-core input dict: [N, CS, H, W] f32 -> dtype-split [N, H, C, W]."""
    mab = np.stack([mao, mbo])  # [2, RW, P*W] bf16
    in_maps = []
    for k in range(NCORES):
        slab = features[:, k * C_SLAB:(k + 1) * C_SLAB].transpose(0, 2, 1, 3)
        in_maps.append({
            "fslab_bf": np.ascontiguousarray(slab[:, :, BF_CS, :]).astype(_BF16),
            "fslab_f8": np.ascontiguousarray(slab[:, :, FP8_CS, :]).astype(_FP8),
            "hmw": np.concatenate([np.asarray(hmask),
                                   np.asarray(mww)], axis=0),
            "recip": recip,
            "mab": mab,
            "zz": zz49,
        })
    return in_maps


def kernel(features: np.ndarray, rois: np.ndarray) -> np.ndarray:
    from concourse.bass_utils import run_bass_kernel_spmd

    features = np.asarray(features, dtype=np.float32)
    rois = np.asarray(rois, dtype=np.float32)

    order, pieces, hmask, mww, recip, mao, mbo, zz49 = _prep(rois)
    nc = _get_program(pieces)

    in_maps = make_in_maps(features, hmask, mww, recip, mao, mbo, zz49)

    res = run_bass_kernel_spmd(nc, in_maps, list(range(NCORES))).results

    result = np.empty((R, C_OUT, C_SLAB), dtype=np.float32)
    for k in range(NCORES):
        result[order, k, :] = res[k]["out"]
    return result.reshape(R, C_OUT, P, P)



# revision 46
# speedup vs baseline: 1.1152x; 1.1152x over previous
"""PSRoIPool (position-sensitive ROI pooling) for Trainium2, 8 NeuronCores.

Problem (hardcoded):
  features [4, 392, 128, 128] f32, rois [512, 5] f32 (batch, x1, y1, x2, y2)
  out [512, 8, 7, 7] f32;  C = C_out(8) * 7 * 7;  spatial_scale = 1/16.

Sharding: by output channel c_out (8 cores). Core k owns feature channels
[49k, 49k+49) of every image (1/8 of the features, read exactly once as
bf16) and computes out[:, k, :, :] for ALL 512 rois.

Algorithm (per core):
  - rois stable-sorted by batch -> exactly 4 windows of 128 rois. A window
    spanning a batch boundary is computed as 2 PSUM-accumulated matmul
    passes whose H-masks are zeroed outside their roi range (host-side).
  - stage 1 (PE, bf16): u[r, pw, w] = sum_h Mh[h, r; ph] * F[h, c_s, w]
    per (window, ph); H-mask is the stationary operand.
  - stage 2 (DVE, one fused pass): custom DVE op MASKED_PREFIX_ANT computes
    P[r, t] = prefix-sum of u[r, t] * Mw[r, t] over the flat (pw, w) axis.
    Bin sums drop out as strided differences at the 128-column boundaries:
      bins[r, ph, 0]    = P[r, 127]
      bins[r, ph, pw>0] = P[r, 128(pw+1)-1] - P[r, 128 pw - 1]
  - out = bins * recip_area (recip precomputed on host, 0 for empty bins).

All masks + reciprocal areas are precomputed on the host and DMAed (they
depend only on the rois and are identical on all 8 cores). Masks are fp8
(0/1 exact, ~1.15 MB total); features are dtype-split: 37 of 49 channels
bf16 + 12 channels fp8 (adds ~1.3e-2 rel err vs the 2e-2 gate, saves 12%
of feature DMA). All input DMAs are consolidated into 5 transfers per
iteration (HWDGE per-DMA overhead ~0.7us dominates small transfers), and
the 4 window outputs leave in a single DMA.
"""

import numpy as np
from contextlib import ExitStack

try:
    import ml_dtypes
    _BF16 = ml_dtypes.bfloat16
    _FP8 = ml_dtypes.float8_e4m3
except ImportError:  # pragma: no cover
    import jax.numpy as _jnp
    _BF16 = _jnp.bfloat16
    _FP8 = _jnp.float8_e4m3

N_IMG, C_FULL, H, W = 4, 392, 128, 128
R = 512
P = 7  # OUT_SIZE == GROUP
C_OUT = 8
C_SLAB = P * P  # 49 channels per core
SCALE = np.float32(0.0625)
NCORES = 8
RW = 128  # rois per window
NWIN = R // RW

OP_NAME = "MASKED_PREFIX_ANT"

# channels stored as fp8 (pw 0-3 of ph 0/2/4): the (0,4) matmul split of each
# ph-pair group. err contribution ~2.6e-2 * sqrt(12/49) ~ 1.3e-2 < 2e-2 gate.
FP8_CS = [0, 1, 2, 3, 14, 15, 16, 17, 28, 29, 30, 31]
BF_CS = [c for c in range(C_SLAB) if c not in FP8_CS]  # 37 channels
N_F8 = len(FP8_CS)
N_BF = len(BF_CS)


# ------------------------------------------------------------- custom DVE op
def _get_custom_op():
    """out[p, k] = sum_{j<=k} in0[p, j] * in1[p, j] (fp32 internal state).

    Registered by appending to concourse.dve_ops.OPS (the per-NEFF DVE
    table is generated from OPS by bass_utils.dve_table_for_ops)."""
    import concourse.dve_ops as D

    for op in D.OPS:
        if op.name == OP_NAME:
            return op

    from concourse.dve_spec import Spec, scan, AluOp, Src0, Src1, lower, _has_src1
    from concourse.dve_uop import DveOpSpec

    def ref(in0, in1, s0, s1, imm2):
        p = in0.astype(np.float32) * in1.astype(np.float32)
        return np.add.accumulate(p, axis=-1).astype(np.float32)

    spec = Spec(body=scan(AluOp.ADD, Src0 * Src1), reference=ref)
    row = max(D._SUB_OPCODE_FOR_NAME.values()) + 1
    assert row < 0x20
    shas = {}
    for ver in ("v3", "v4"):
        s = DveOpSpec(name=OP_NAME, opcode=row,
                      uops=lower(spec, ver=ver), rd1_en=_has_src1(spec))
        shas[ver] = s.sha(ver)
    op = D.DveOp(OP_NAME, spec, subdim=False, uops_sha=shas)
    D.OPS.append(op)
    D.CUSTOM_DVE_SPECS[op.name] = op.spec
    D._SUB_OPCODE_FOR_NAME[op.name] = row
    return op


# ---------------------------------------------------------------- host math
def _bounds(rois_sorted: np.ndarray):
    """Bit-exact f32 mirror of the reference coordinate transform."""
    r = rois_sorted.astype(np.float32)
    one = np.float32(1.0)
    rsw = np.round(r[:, 1]) * SCALE
    rsh = np.round(r[:, 2]) * SCALE
    rew = (np.round(r[:, 3]) + one) * SCALE
    reh = (np.round(r[:, 4]) + one) * SCALE
    roi_w = np.maximum(rew - rsw, np.float32(0.1))
    roi_h = np.maximum(reh - rsh, np.float32(0.1))
    bin_w = (roi_w / np.float32(P)).astype(np.float32)
    bin_h = (roi_h / np.float32(P)).astype(np.float32)
    p = np.arange(P, dtype=np.float32)
    hs = np.clip(np.floor(p[None, :] * bin_h[:, None] + rsh[:, None]), 0, H)
    he = np.clip(np.ceil((p[None, :] + one) * bin_h[:, None] + rsh[:, None]), 0, H)
    ws = np.clip(np.floor(p[None, :] * bin_w[:, None] + rsw[:, None]), 0, W)
    we = np.clip(np.ceil((p[None, :] + one) * bin_w[:, None] + rsw[:, None]), 0, W)
    return hs, he, ws, we  # [R, P] f32 (integer-valued)


def _plan(batch: np.ndarray):
    """Pack rois into NWIN windows of RW, minimising matmul streams: pull
    whole-window pure-batch groups first, then chunk the remainders.
    Returns (order [R], pieces: per-window ((b, c0, c1), ...))."""
    groups = [np.nonzero(batch == b)[0] for b in range(N_IMG)]
    pure, rem = [], []
    for idx in groups:
        n_pure = len(idx) // RW
        for i in range(n_pure):
            pure.append(idx[i * RW:(i + 1) * RW])
        rem.append(idx[n_pure * RW:])
    rem = np.concatenate([r for r in rem if len(r)]) if any(
        len(r) for r in rem) else np.empty(0, np.int64)
    order = np.concatenate(pure + ([rem] if len(rem) else []))
    assert len(order) == R
    sb = batch[order]
    pieces = []
    for w in range(NWIN):
        seg = sb[w * RW:(w + 1) * RW]
        ps = []
        start = 0
        for i in range(1, RW + 1):
            if i == RW or seg[i] != seg[start]:
                ps.append((int(seg[start]), start, i))
                start = i
        pieces.append(tuple(ps))
    return order, tuple(pieces)


def _prep(rois: np.ndarray):
    batch = rois[:, 0].astype(np.int32)
    order, pieces = _plan(batch)
    rs = rois[order]
    hs, he, ws, we = _bounds(rs)

    harange = np.arange(H, dtype=np.float32)
    warange = np.arange(W, dtype=np.float32)

    # hmask streams: one per (window, piece): [h, ph, r] zeroed outside piece
    hm_list = []
    for w, ps in enumerate(pieces):
        sl = slice(w * RW, (w + 1) * RW)
        hsw, hew = hs[sl], he[sl]  # [RW, P]
        m = ((harange[:, None, None] >= hsw.T[None, :, :])
             & (harange[:, None, None] < hew.T[None, :, :]))  # [H, P, RW]
        for (b, c0, c1) in ps:
            mm = np.zeros((H, P, RW), dtype=np.float32)
            mm[:, :, c0:c1] = m[:, :, c0:c1]
            hm_list.append(mm.reshape(H, P * RW))
    hmask = np.asarray(np.stack(hm_list), dtype=_FP8)  # [NS, 128, 896]

    # W-interval mask per window: [r, pw, w]
    mw = ((warange[None, None, :] >= ws[:, :, None])
          & (warange[None, None, :] < we[:, :, None])).astype(np.float32)
    mww = np.asarray(mw.reshape(NWIN, RW, P * W), dtype=_FP8)

    # open-ended masks for the Pool (gpsimd) scan path on window 0:
    # (w >= max(bound, 1)) so the (u + state) * m recurrence resets at each
    # 128-col channel start; the dropped w=0 term is re-added as zz * u[.,0]
    w0 = slice(0, RW)
    ma = (warange[None, None, :] >= np.maximum(ws[w0], 1.0)[:, :, None])
    mb = (warange[None, None, :] >= np.maximum(we[w0], 1.0)[:, :, None])
    mao = np.asarray(ma.reshape(RW, P * W).astype(np.float32), dtype=_BF16)
    mbo = np.asarray(mb.reshape(RW, P * W).astype(np.float32), dtype=_BF16)
    zz = ((ws[w0] == 0).astype(np.float32)
          - (we[w0] == 0).astype(np.float32))  # [RW, P] per pw
    # zz indexed by cs = ph*7+pw -> tile [RW, 49]
    zz49 = np.repeat(zz[:, None, :], P, axis=1).reshape(RW, C_SLAB)
    zz49 = zz49.astype(np.float32)

    # reciprocal area, 0 where empty  [NWIN, RW, 49] (cs = ph*7+pw)
    ah = he - hs  # [R, P]
    aw = we - ws
    area = ah[:, :, None] * aw[:, None, :]  # [R, ph, pw]
    recip = np.where(area > 0, np.float32(1.0) / np.maximum(area, 1.0),
                     np.float32(0.0)).astype(np.float32)
    recip = recip.reshape(NWIN, RW, C_SLAB)

    return order, pieces, hmask, mww, recip, mao, mbo, zz49


# ---------------------------------------------------------------- device IR
def build_program(pieces, repeat=1, dma_once=False, tiny_scan=False,
                  fat_scan=False, first_piece_only=False, pool_gis=()):
    """pool_gis: window-0 group indices whose scans would run on GPSIMD
    (Pool) instead of DVE. MUST stay () — the neuronx backend cannot
    compile tensor_tensor_scan on the Pool engine (verified empirically:
    even a minimal Pool-scan program fails in the compile hook), which is
    why stage 2 uses the custom DVE op in the first place."""
    import concourse.bass as bass
    import concourse.tile as tile
    from concourse import bacc, mybir

    f32 = mybir.dt.float32
    bf16 = mybir.dt.bfloat16
    fp8 = mybir.dt.float8e4
    Alu = mybir.AluOpType
    op = _get_custom_op()

    nstream = sum(len(ps) for ps in pieces)

    nc = bacc.Bacc("TRN2", target_bir_lowering=False, debug=False,
                   num_devices=NCORES)

    # feature slab split by storage dtype; [N, H, C, W] so each partition
    # line is C*W contiguous bytes
    fslab_bf = nc.dram_tensor("fslab_bf", [N_IMG, H, N_BF, W], bf16,
                              kind="ExternalInput").ap()
    fslab_f8 = nc.dram_tensor("fslab_f8", [N_IMG, H, N_F8, W], fp8,
                              kind="ExternalInput").ap()
    # NOTE: merging hmask+mww into one DMA was tried and REGRESSED (26.6 ->
    # 30.1us): the w-mask for window 0's scan then only lands after the full
    # combined transfer, stalling the DVE pipeline head. Keep them separate.
    hmask = nc.dram_tensor("hmask", [nstream, H, P * RW], fp8,
                           kind="ExternalInput").ap()
    mww = nc.dram_tensor("mww", [NWIN, RW, P * W], fp8,
                         kind="ExternalInput").ap()
    # open-ended masks (mao; mbo) + zz edge term for the Pool scan of win 0
    mab = nc.dram_tensor("mab", [2, RW, P * W], bf16,
                         kind="ExternalInput").ap()
    zz = nc.dram_tensor("zz", [RW, C_SLAB], f32, kind="ExternalInput").ap()
    recip = nc.dram_tensor("recip", [NWIN, RW, C_SLAB], f32,
                           kind="ExternalInput").ap()
    out = nc.dram_tensor("out", [R, C_SLAB], f32, kind="ExternalOutput").ap()

    with tile.TileContext(nc) as tc, ExitStack() as ctx:
        fpool = ctx.enter_context(tc.tile_pool(name="fs", bufs=2))
        mpool = ctx.enter_context(tc.tile_pool(name="masks", bufs=2))
        # bufs=3 on scratch: the DVE scan double-buffers ahead of the
        # ACT/Pool bins extraction; 2 bufs was only needed to fit the
        # (disabled) pool_gis path's extra tiles in SBUF
        spool = ctx.enter_context(tc.tile_pool(name="scratch", bufs=3))
        upool = ctx.enter_context(tc.tile_pool(name="ucopy", bufs=2))
        bpool = ctx.enter_context(tc.tile_pool(name="bins", bufs=2))
        opool = ctx.enter_context(tc.tile_pool(name="outw", bufs=2))
        # u2 tiles are [128, 14, 128] f32 = 3.5 PSUM banks; 2 bufs = 7 of 8
        psum = ctx.enter_context(tc.tile_pool(name="ps", bufs=2, space="PSUM"))

        for _rep in range(repeat):
            if _rep == 0 or not dma_once:
                # one consolidated DMA per tensor: DMA-queue (HWDGE) churn
                # dominates small transfers, so batch everything
                fsall = fpool.tile([128, N_IMG, N_BF, W], bf16, tag="fsall")
                nc.sync.dma_start(
                    out=fsall[:],
                    in_=bass.AP(tensor=fslab_bf.tensor, offset=0,
                                ap=[[N_BF * W, H], [H * N_BF * W, N_IMG],
                                    [W, N_BF], [1, W]]))
                fsall8 = fpool.tile([128, N_IMG, N_F8, W], fp8, tag="fsall8")
                nc.sync.dma_start(
                    out=fsall8[:],
                    in_=bass.AP(tensor=fslab_f8.tensor, offset=0,
                                ap=[[N_F8 * W, H], [H * N_F8 * W, N_IMG],
                                    [W, N_F8], [1, W]]))
                fs = [(fsall[:, b], fsall8[:, b]) for b in range(N_IMG)]

            hmall = mpool.tile([128, nstream, P * RW], fp8, tag="hmall")
            nc.sync.dma_start(
                out=hmall[:],
                in_=bass.AP(tensor=hmask.tensor, offset=0,
                            ap=[[P * RW, H], [H * P * RW, nstream],
                                [1, P * RW]]))
            mwall = mpool.tile([128, NWIN, P * W], fp8, tag="mwall")
            nc.sync.dma_start(
                out=mwall[:],
                in_=bass.AP(tensor=mww.tensor, offset=0,
                            ap=[[P * W, RW], [RW * P * W, NWIN], [1, P * W]]))
            rcall = mpool.tile([128, NWIN, C_SLAB], f32, tag="rcall")
            nc.sync.dma_start(
                out=rcall[:],
                in_=bass.AP(tensor=recip.tensor, offset=0,
                            ap=[[C_SLAB, RW], [RW * C_SLAB, NWIN],
                                [1, C_SLAB]]))
            if pool_gis:
                mabt = mpool.tile([128, 2, P * W], bf16, tag="mabt")
                nc.sync.dma_start(
                    out=mabt[:],
                    in_=bass.AP(tensor=mab.tensor, offset=0,
                                ap=[[P * W, RW], [RW * P * W, 2],
                                    [1, P * W]]))
                zzt = mpool.tile([128, C_SLAB], f32, tag="zzt")
                nc.sync.dma_start(
                    out=zzt[:],
                    in_=bass.AP(tensor=zz.tensor, offset=0,
                                ap=[[C_SLAB, RW], [1, C_SLAB]]))
            outw = opool.tile([128, NWIN, C_SLAB], f32, tag="outw")

            sidx = 0
            for win in range(NWIN):
                ps = pieces[win]
                hm_off = []
                for _ in ps:
                    hm_off.append(hmall.offset + sidx * P * RW)
                    sidx += 1
                mw_off = mwall.offset + win * P * W
                bins = bpool.tile([128, C_SLAB], f32, tag="bins")

                # ph groups: pairs (0,1),(2,3),(4,5) + single (6,). One fused
                # scan per group over the PSUM tile; matmul column-splits are
                # PSUM-bank-aligned AND split at the ph boundary (row 7):
                # rows 0-6 use ph a's H-mask, rows 7-13 ph b's.
                for gi, grp in enumerate(((0, 1), (2, 3), (4, 5), (6,))):
                    pa = grp[0]
                    nrow = 7 * len(grp)
                    base = pa * P  # contiguous channels base+0 .. base+nrow-1
                    u = psum.tile([128, 2 * P, W], f32, tag="u2")
                    if len(grp) == 2:
                        splits = ((0, 4, grp[0]), (4, 7, grp[0]),
                                  (7, 8, grp[1]), (8, 12, grp[1]),
                                  (12, 14, grp[1]))
                    else:
                        splits = ((0, 4, grp[0]), (4, 7, grp[0]))
                    g = pa // 2
                    ps_mm = ps[:1] if first_piece_only else ps
                    for (n0, n1, p_) in splits:
                        # rhs rows in the dtype-split storage: the (0,4)
                        # split of each pair group reads the fp8 slab
                        if len(grp) == 2 and n0 == 0:
                            def rhs_of(b):
                                return bass.AP(
                                    tensor=fsall8.tensor,
                                    offset=fsall8.offset
                                    + (b * N_F8 + g * 4) * W,
                                    ap=[fsall8.ap[0], [W, 4], [1, W]])
                        else:
                            row0 = (g * 10 + n0 - 4) if len(grp) == 2 \
                                else (30 + n0)
                            def rhs_of(b, row0=row0, nr=n1 - n0):
                                return bass.AP(
                                    tensor=fsall.tensor,
                                    offset=fsall.offset
                                    + (b * N_BF + row0) * W,
                                    ap=[fsall.ap[0], [W, nr], [1, W]])
                        for i, (b, c0, c1) in enumerate(ps_mm):
                            nc.tensor.matmul(
                                out=u[:, n0:n1, :],
                                lhsT=bass.AP(
                                    tensor=hmall.tensor,
                                    offset=hm_off[i] + p_ * RW,
                                    ap=[hmall.ap[0], [1, RW]]),
                                rhs=rhs_of(b),
                                start=(i == 0), stop=(i == len(ps_mm) - 1))
                    if (win == 0 and gi in pool_gis
                            and not (tiny_scan or fat_scan)):
                        # offloaded group: scans on the (otherwise idle)
                        # GPSIMD. ACT first copies u PSUM->SBUF f32 so the
                        # PSUM buffer is released at ACT speed and the slow
                        # Pool scans never stall the PE's PSUM rotation.
                        # Two open-ended (u + state) * m recurrences per ph;
                        # bin = A_end - B_end + zz * u[., 0] at the fixed
                        # per-channel end columns (state resets at each
                        # 128-col start since the masks are 0 at w=0).
                        ucp = upool.tile([128, 2 * P * W], f32, tag="ucp")
                        u_flat = bass.AP(tensor=u.tensor, offset=u.offset,
                                         ap=[u.ap[0], [1, nrow * W]])
                        nc.scalar.copy(ucp[:, 0:nrow * W], u_flat)
                        preA = spool.tile([128, 2 * P * W], f32, tag="preA")
                        preB = spool.tile([128, 2 * P * W], f32, tag="preB")
                        for phi in range(len(grp)):
                            u_sl = bass.AP(tensor=ucp.tensor,
                                           offset=ucp.offset + phi * P * W,
                                           ap=[ucp.ap[0], [1, P * W]])
                            for pre_t, moff in ((preA, 0), (preB, P * W)):
                                nc.gpsimd.tensor_tensor_scan(
                                    out=bass.AP(
                                        tensor=pre_t.tensor,
                                        offset=pre_t.offset + phi * P * W,
                                        ap=[pre_t.ap[0], [1, P * W]]),
                                    data0=u_sl,
                                    data1=bass.AP(
                                        tensor=mabt.tensor,
                                        offset=mabt.offset + moff,
                                        ap=[mabt.ap[0], [1, P * W]]),
                                    initial=0.0,
                                    op0=Alu.add, op1=Alu.mult)
                        cs0 = base
                        hiA = bass.AP(tensor=preA.tensor,
                                      offset=preA.offset + W - 1,
                                      ap=[preA.ap[0], [W, nrow]])
                        hiB = bass.AP(tensor=preB.tensor,
                                      offset=preB.offset + W - 1,
                                      ap=[preB.ap[0], [W, nrow]])
                        nc.gpsimd.tensor_tensor(out=bins[:, cs0:cs0 + nrow],
                                                in0=hiA, in1=hiB,
                                                op=Alu.subtract)
                        u0 = bass.AP(tensor=ucp.tensor, offset=ucp.offset,
                                     ap=[ucp.ap[0], [W, nrow]])
                        zsl = bass.AP(tensor=zzt.tensor,
                                      offset=zzt.offset + cs0,
                                      ap=[zzt.ap[0], [1, nrow]])
                        ztmp = spool.tile([128, 2 * P], f32, tag="ztmp")
                        nc.gpsimd.tensor_tensor(out=ztmp[:, 0:nrow],
                                                in0=u0, in1=zsl, op=Alu.mult)
                        nc.gpsimd.tensor_tensor(out=bins[:, cs0:cs0 + nrow],
                                                in0=bins[:, cs0:cs0 + nrow],
                                                in1=ztmp[:, 0:nrow],
                                                op=Alu.add)
                        continue
                    pre = spool.tile([128, (4 if fat_scan else 2) * P * W],
                                     f32, tag="pre")
                    if fat_scan:
                        # DVE-rate probe: stream u through the scan twice via
                        # a [0,2] broadcast — results in the first copy are
                        # unchanged, DVE scan time doubles.
                        u_fat = bass.AP(tensor=u.tensor, offset=u.offset,
                                        ap=[u.ap[0], [0, 2], [1, nrow * W]])
                        m_fat = bass.AP(tensor=mwall.tensor, offset=mw_off,
                                        ap=[mwall.ap[0],
                                            [0, 4 if len(grp) == 2 else 2],
                                            [1, P * W]])
                        nc.vector._custom_dve(op, out=pre[:, 0:2 * nrow * W],
                                              in0=u_fat, in1=m_fat)
                    elif tiny_scan:
                        # timing probe: scan only the 14 columns that are
                        # later read (positions k*W + W-1), deps preserved
                        pre_s = bass.AP(tensor=pre.tensor,
                                        offset=pre.offset + W - 1,
                                        ap=[pre.ap[0], [W, nrow]])
                        u_s = bass.AP(tensor=u.tensor, offset=u.offset + W - 1,
                                      ap=[u.ap[0], [W, nrow]])
                        if len(grp) == 2:
                            m_s = bass.AP(tensor=mwall.tensor,
                                          offset=mw_off + W - 1,
                                          ap=[mwall.ap[0], [0, 2], [W, P]])
                        else:
                            m_s = bass.AP(tensor=mwall.tensor,
                                          offset=mw_off + W - 1,
                                          ap=[mwall.ap[0], [W, P]])
                        nc.vector._custom_dve(op, out=pre_s, in0=u_s, in1=m_s)
                    else:
                        u_flat = bass.AP(tensor=u.tensor, offset=u.offset,
                                         ap=[u.ap[0], [1, nrow * W]])
                        if len(grp) == 2:
                            # mask streams twice (same [r, pw, w] mask per ph)
                            m_in = bass.AP(tensor=mwall.tensor, offset=mw_off,
                                           ap=[mwall.ap[0], [0, 2],
                                               [1, P * W]])
                        else:
                            m_in = bass.AP(tensor=mwall.tensor, offset=mw_off,
                                           ap=[mwall.ap[0], [1, P * W]])
                        nc.vector._custom_dve(op, out=pre[:, 0:nrow * W],
                                              in0=u_flat, in1=m_in)
                    cs0 = base
                    nc.scalar.copy(bins[:, cs0:cs0 + 1], pre[:, W - 1:W])
                    hi = bass.AP(tensor=pre.tensor, offset=pre.offset + 2 * W - 1,
                                 ap=[pre.ap[0], [W, nrow - 1]])
                    lo = bass.AP(tensor=pre.tensor, offset=pre.offset + W - 1,
                                 ap=[pre.ap[0], [W, nrow - 1]])
                    nc.gpsimd.tensor_tensor(out=bins[:, cs0 + 1:cs0 + nrow],
                                            in0=hi, in1=lo, op=Alu.subtract)

                rct = bass.AP(tensor=rcall.tensor,
                              offset=rcall.offset + win * C_SLAB,
                              ap=[rcall.ap[0], [1, C_SLAB]])
                ow = bass.AP(tensor=outw.tensor,
                             offset=outw.offset + win * C_SLAB,
                             ap=[outw.ap[0], [1, C_SLAB]])
                nc.gpsimd.tensor_tensor(out=ow, in0=bins[:], in1=rct,
                                        op=Alu.mult)

            # single consolidated output DMA: SBUF [r, win, cs] -> out rows
            nc.sync.dma_start(
                out=bass.AP(tensor=out.tensor, offset=0,
                            ap=[[C_SLAB, RW], [RW * C_SLAB, NWIN],
                                [1, C_SLAB]]),
                in_=outw[:])

    nc.compile()
    return nc


_PROG_CACHE = {}


def _get_program(pieces, repeat=1):
    key = (pieces, repeat)
    if key not in _PROG_CACHE:
        _PROG_CACHE[key] = build_program(pieces, repeat=repeat)
    return _PROG_CACHE[key]


# ---------------------------------------------------------------- entrypoint
def make_in_maps(features: np.ndarray, hmask, mww, recip, mao, mbo, zz49):
    """Per-core input dict: [N, CS, H, W] f32 -> dtype-split [N, H, C, W]."""
    mab = np.stack([mao, mbo])  # [2, RW, P*W] bf16
    in_maps = []
    for k in range(NCORES):
        slab = features[:, k * C_SLAB:(k + 1) * C_SLAB].transpose(0, 2, 1, 3)
        in_maps.append({
            "fslab_bf": np.ascontiguousarray(slab[:, :, BF_CS, :]).astype(_BF16),
            "fslab_f8": np.ascontiguousarray(slab[:, :, FP8_CS, :]).astype(_FP8),
            "hmask": hmask,
            "mww": mww,
            "recip": recip,
            "mab": mab,
            "zz": zz49,
        })
    return in_maps


def kernel(features: np.ndarray, rois: np.ndarray) -> np.ndarray:
    from concourse.bass_utils import run_bass_kernel_spmd

    features = np.asarray(features, dtype=np.float32)
    rois = np.asarray(rois, dtype=np.float32)

    order, pieces, hmask, mww, recip, mao, mbo, zz49 = _prep(rois)
    nc = _get_program(pieces)

    in_maps = make_in_maps(features, hmask, mww, recip, mao, mbo, zz49)

    res = run_bass_kernel_spmd(nc, in_maps, list(range(NCORES))).results

    result = np.empty((R, C_OUT, C_SLAB), dtype=np.float32)
    for k in range(NCORES):
        result[order, k, :] = res[k]["out"]
    return result.reshape(R, C_OUT, P, P)

